# revision 8
# baseline (speedup 1.0000x reference)
"""Trainium2 Bass kernel for CudaMorphUnpool2D (max-unpool scatter + 3x3 dilation).

Strategy (v8):
  - 1024 (b,c) planes sharded 128/core across 8 NeuronCores (fully data parallel).
  - Host pre-computes, per pooled cell, its scatter target as a flat index into a
    chunked, parity-split canvas layout (64 chunks/plane, each chunk = 2 canvas
    pair-rows laid out as 4 quadrant segments [ee|oe|eo|oo] of 130 cols w/ guard
    slots).  Cells overwritten by a later raster writer (reference last-writer-
    wins semantics) are dropped, making indices collision-free; the surviving
    (value, index) pairs are compacted to NIDX slots per chunk (the per-index
    cost dominates GPSIMD LocalScatter: ~173 + 0.82/elem + 3.1/idx ns).
  - Device: GPSIMD local_scatter builds the canvas (zeroing dst = free guards /
    empty cells), DVE does the separable 3x3 max (colmax slab-local; rowmax lags
    one slab, reading boundary cm rows of neighbouring slabs), ACT makes the
    4B-aligned shifted copies.  16 fine slabs keep the pipeline ramp short.
  - Planar [E|O] column output; host deinterleaves + casts fp16 -> fp32.
"""
import os
import sys
import numpy as np
from contextlib import ExitStack

H, W = 256, 256
HP, WP = 128, 128
NCORES = 8
PPC = 128               # planes per core

NCHUNK = 64             # canvas chunks per plane (2 pair-rows each)
NIDX = 168              # compacted source slots per chunk (max valid = 163)
SEG = 130               # quadrant segment width (128 + 2 guard slots)
PAIRW = 4 * SEG         # 520 elements per canvas pair-row
NELEM = 2 * PAIRW       # 1040 elements per chunk
NSLAB = 16              # 16 slabs x 4 chunks
CPS = NCHUNK // NSLAB   # chunks per slab = 4
RPS = 2 * CPS           # pair-rows per slab = 8

for _p in ("/opt/trn_rl_repo", "/root/.axon_site/_ro/trn_rl_repo"):
    if os.path.isdir(_p) and _p not in sys.path:
        sys.path.append(_p)


def _build_nc():
    import concourse.bass as bass  # noqa: F401
    import concourse.tile as tile
    from concourse import bacc, mybir

    dt = mybir.dt.float16
    AO = mybir.AluOpType

    nc = bacc.Bacc("TRN2", target_bir_lowering=False, debug=False)
    v_in = nc.dram_tensor("vals", [PPC, NCHUNK, NIDX], dt, kind="ExternalInput").ap()
    ix_in = nc.dram_tensor("ix", [PPC, NCHUNK, NIDX], mybir.dt.int16,
                           kind="ExternalInput").ap()
    o_out = nc.dram_tensor("out", [PPC, H, W], dt, kind="ExternalOutput").ap()

    with tile.TileContext(nc) as tc, ExitStack() as ctx:
        pv = ctx.enter_context(tc.tile_pool(name="pv", bufs=3))
        pix = ctx.enter_context(tc.tile_pool(name="pix", bufs=3))
        pcv = ctx.enter_context(tc.tile_pool(name="pcv", bufs=3))
        psh = ctx.enter_context(tc.tile_pool(name="psh", bufs=3))
        pcm = ctx.enter_context(tc.tile_pool(name="pcm", bufs=3))
        pp = ctx.enter_context(tc.tile_pool(name="pp", bufs=2))
        pout = ctx.enter_context(tc.tile_pool(name="pout", bufs=2))

        cms = {}

        def scatter_slab(s):
            """local_scatter chunks 4s..4s+3 into one flat canvas tile."""
            VX = pv.tile([128, CPS, NIDX], dt, tag="VX")
            IX = pix.tile([128, CPS, NIDX], mybir.dt.int16, tag="IX")
            nc.sync.dma_start(VX[:], v_in[:, CPS * s:CPS * (s + 1), :])
            nc.sync.dma_start(IX[:], ix_in[:, CPS * s:CPS * (s + 1), :])
            CV = pcv.tile([128, CPS * NELEM], dt, tag="CV")
            for c in range(CPS):
                nc.gpsimd.local_scatter(
                    CV[:, c * NELEM:(c + 1) * NELEM], VX[:, c, :], IX[:, c, :],
                    channels=128, num_elems=NELEM, num_idxs=NIDX)
            return CV

        def colmax_slab(s, CV):
            """cm tiles [128,RPS,128]: row r <-> canvas pair-row RPS*s+r."""
            v = CV[:].rearrange("p (a g w) -> p a g w", a=RPS, g=4, w=SEG)
            E_e = v[:, :, 0, 0:128]
            E_o = v[:, :, 1, 0:128]
            O_e = v[:, :, 2, 2:130]
            O_o = v[:, :, 3, 2:130]
            sh_ee = psh.tile([128, RPS, 128], dt, tag="sh_ee")
            sh_eo = psh.tile([128, RPS, 128], dt, tag="sh_eo")
            sh_oe = psh.tile([128, RPS, 128], dt, tag="sh_oe")
            sh_oo = psh.tile([128, RPS, 128], dt, tag="sh_oo")
            nc.scalar.copy(sh_ee[:], v[:, :, 0, 1:129])
            nc.scalar.copy(sh_oe[:], v[:, :, 1, 1:129])
            nc.scalar.copy(sh_eo[:], v[:, :, 2, 1:129])
            nc.scalar.copy(sh_oo[:], v[:, :, 3, 1:129])
            cm_eE = pcm.tile([128, RPS, 128], dt, tag="cm_eE")
            cm_eO = pcm.tile([128, RPS, 128], dt, tag="cm_eO")
            cm_oE = pcm.tile([128, RPS, 128], dt, tag="cm_oE")
            cm_oO = pcm.tile([128, RPS, 128], dt, tag="cm_oO")
            P_e = pp.tile([128, RPS, 128], dt, tag="P_e")
            P_o = pp.tile([128, RPS, 128], dt, tag="P_o")
            nc.vector.tensor_tensor(P_e[:], E_e, O_e, AO.max)
            nc.vector.tensor_tensor(P_o[:], E_o, O_o, AO.max)
            nc.vector.tensor_tensor(cm_eE[:], P_e[:], sh_eo[:], AO.max)
            nc.vector.tensor_tensor(cm_eO[:], P_e[:], sh_ee[:], AO.max)
            nc.vector.tensor_tensor(cm_oE[:], P_o[:], sh_oo[:], AO.max)
            nc.vector.tensor_tensor(cm_oO[:], P_o[:], sh_oe[:], AO.max)
            cms[s] = (cm_eE, cm_eO, cm_oE, cm_oO)

        def rowmax_slab(s):
            """out rows [16s,16s+16): even r -> max(cm_o[r-1], S[r]),
            odd r -> max(S[r], cm_e[r+1]); S[r] = max(cm_e[r], cm_o[r])."""
            cm_eE, cm_eO, cm_oE, cm_oO = cms[s]
            prev = cms.get(s - 1)
            nxt = cms.get(s + 1)
            L = RPS - 1
            out_t = pout.tile([128, RPS, 2, 2, 128], dt, tag="out_t")
            S_E = pp.tile([128, RPS, 128], dt, tag="S_E")
            S_O = pp.tile([128, RPS, 128], dt, tag="S_O")
            nc.vector.tensor_tensor(S_E[:], cm_eE[:], cm_oE[:], AO.max)
            nc.vector.tensor_tensor(S_O[:], cm_eO[:], cm_oO[:], AO.max)
            nc.vector.tensor_tensor(out_t[:, 1:, 0, 0, :], S_E[:, 1:, :],
                                    cm_oE[:, 0:L, :], AO.max)
            nc.vector.tensor_tensor(out_t[:, 1:, 0, 1, :], S_O[:, 1:, :],
                                    cm_oO[:, 0:L, :], AO.max)
            nc.vector.tensor_tensor(out_t[:, 0:L, 1, 0, :], S_E[:, 0:L, :],
                                    cm_eE[:, 1:, :], AO.max)
            nc.vector.tensor_tensor(out_t[:, 0:L, 1, 1, :], S_O[:, 0:L, :],
                                    cm_eO[:, 1:, :], AO.max)
            if prev is None:
                nc.scalar.copy(out_t[:, 0, 0, 0, :], S_E[:, 0, :])
                nc.scalar.copy(out_t[:, 0, 0, 1, :], S_O[:, 0, :])
            else:
                nc.vector.tensor_tensor(out_t[:, 0, 0, 0, :], S_E[:, 0, :],
                                        prev[2][:, L, :], AO.max)
                nc.vector.tensor_tensor(out_t[:, 0, 0, 1, :], S_O[:, 0, :],
                                        prev[3][:, L, :], AO.max)
            if nxt is None:
                nc.scalar.copy(out_t[:, L, 1, 0, :], S_E[:, L, :])
                nc.scalar.copy(out_t[:, L, 1, 1, :], S_O[:, L, :])
            else:
                nc.vector.tensor_tensor(out_t[:, L, 1, 0, :], S_E[:, L, :],
                                        nxt[0][:, 0, :], AO.max)
                nc.vector.tensor_tensor(out_t[:, L, 1, 1, :], S_O[:, L, :],
                                        nxt[1][:, 0, :], AO.max)

            ov = o_out[:, 16 * s:16 * s + 16, :].rearrange(
                "p (r two) (x c) -> p r two x c", two=2, x=2)
            nc.sync.dma_start(ov, out_t[:])

        CV = scatter_slab(0)
        colmax_slab(0, CV)
        for s in range(1, NSLAB):
            CV = scatter_slab(s)
            colmax_slab(s, CV)
            rowmax_slab(s - 1)
        rowmax_slab(NSLAB - 1)

    nc.compile()
    return nc


_NC_CACHE = {}


def _get_nc():
    if "nc" not in _NC_CACHE:
        _NC_CACHE["nc"] = _build_nc()
    return _NC_CACHE["nc"]


def prepare_inputs(f, p):
    """Host prep: compacted collision-free (value, index) scatter lists.

    Returns (vals, idxs): both [N, NCHUNK, NIDX] (fp16 / int16).
    """
    N = f.shape[0] * f.shape[1]
    f2 = f.reshape(N, HP, WP)
    p2 = p.reshape(N, HP, WP).astype(np.int32)

    base = (np.arange(HP, dtype=np.int32)[:, None] * (2 * W)
            + np.arange(WP, dtype=np.int32)[None, :] * 2)
    d = p2 - base[None]
    dy = d >> 8
    dx = d & 255

    def sh(a, di, dj):
        out = np.full_like(a, -9)
        si0, si1 = max(di, 0), HP + min(di, 0)
        sj0, sj1 = max(dj, 0), WP + min(dj, 0)
        out[:, si0 - di:si1 - di, sj0 - dj:sj1 - dj] = a[:, si0:si1, sj0:sj1]
        return out

    dyR, dxR = sh(dy, 0, 1), sh(dx, 0, 1)
    dyD0, dxD0 = sh(dy, 1, 0), sh(dx, 1, 0)
    dyDm, dxDm = sh(dy, 1, -1), sh(dx, 1, -1)
    dyDp, dxDp = sh(dy, 1, 1), sh(dx, 1, 1)
    killed = ((dx == 2) & (dxR == 0) & (dyR == dy)) | ((dy == 2) & (
        ((dx == 0) & (dyDm == 0) & (dxDm == 2)) |
        ((dyD0 == 0) & (dxD0 == dx)) |
        ((dx == 2) & (dyDp == 0) & (dxDp == 0))))
    # Non-positive values can never win a window max (every 3x3 window of the
    # canvas contains an empty 0 cell), and their overwrite effect is already
    # captured by `killed` -- drop them from the scatter entirely.
    fp = f2.astype(np.float16)
    drop = killed | (fp <= 0)

    y = 2 * np.arange(HP, dtype=np.int32)[None, :, None] + dy
    x = 2 * np.arange(WP, dtype=np.int32)[None, None, :] + dx
    a = y >> 1
    seg = (x & 1) * 2 + (y & 1)              # [ee, oe, eo, oo]
    local_idx = (a & 1) * PAIRW + seg * SEG + np.where(x & 1 == 1, 2, 0) + (x >> 1)
    chunk_of = a >> 1

    # slot layout before compaction: 3 source rows (2t-1, 2t, 2t+1) x 128
    idx384 = np.full((N, NCHUNK, 384), -1, dtype=np.int16)
    val384 = np.zeros((N, NCHUNK, 384), dtype=np.float16)
    tt = np.arange(NCHUNK)
    r0 = np.maximum(0, 2 * tt - 1)
    for rloc in range(3):
        iv = r0 + rloc
        li = local_idx[:, iv, :]
        co = chunk_of[:, iv, :]
        kk = drop[:, iv, :]
        idx384[:, :, rloc * 128:(rloc + 1) * 128] = np.where(
            (co == tt[None, :, None]) & ~kk, li, -1).astype(np.int16)
        val384[:, :, rloc * 128:(rloc + 1) * 128] = fp[:, iv, :]

    # compact: valid slots first (stable), then truncate to NIDX
    invalid = idx384 < 0
    nvalid = (~invalid).sum(axis=-1)
    assert nvalid.max() <= NIDX, f"NIDX too small: need {nvalid.max()}"
    order = np.argsort(invalid, axis=-1, kind="stable")
    idxs = np.take_along_axis(idx384, order, axis=-1)[:, :, :NIDX]
    vals = np.take_along_axis(val384, order, axis=-1)[:, :, :NIDX]
    return np.ascontiguousarray(vals), np.ascontiguousarray(idxs)


def postprocess(out_planar):
    """[K, 256, 256] planar rows [E|O] -> interleaved columns, fp32."""
    res = np.empty(out_planar.shape, dtype=np.float32)
    res[..., 0::2] = out_planar[..., 0:128]
    res[..., 1::2] = out_planar[..., 128:256]
    return res


def kernel(**inputs):
    f = np.asarray(inputs["f"])
    p = np.asarray(inputs["provenance"])
    B, C = f.shape[:2]
    assert f.shape == (B, C, HP, WP) and B * C == NCORES * PPC

    vals, idxs = prepare_inputs(f, p)

    nc = _get_nc()
    from concourse.bass_utils import run_bass_kernel_spmd
    in_maps = [{"vals": vals[k * PPC:(k + 1) * PPC], "ix": idxs[k * PPC:(k + 1) * PPC]}
               for k in range(NCORES)]
    res = run_bass_kernel_spmd(nc, in_maps, core_ids=list(range(NCORES)))
    out = np.concatenate([postprocess(res.results[k]["out"]) for k in range(NCORES)],
                         axis=0)
    return out.reshape(B, C, H, W)


# revision 9
# speedup vs baseline: 1.0359x; 1.0359x over previous
"""Trainium2 Bass kernel for CudaMorphUnpool2D (max-unpool scatter + 3x3 dilation).

Strategy (v8):
  - 1024 (b,c) planes sharded 128/core across 8 NeuronCores (fully data parallel).
  - Host pre-computes, per pooled cell, its scatter target as a flat index into a
    chunked, parity-split canvas layout (64 chunks/plane, each chunk = 2 canvas
    pair-rows laid out as 4 quadrant segments [ee|oe|eo|oo] of 130 cols w/ guard
    slots).  Cells overwritten by a later raster writer (reference last-writer-
    wins semantics) are dropped, making indices collision-free; the surviving
    (value, index) pairs are compacted to NIDX slots per chunk (the per-index
    cost dominates GPSIMD LocalScatter: ~173 + 0.82/elem + 3.1/idx ns).
  - Device: GPSIMD local_scatter builds the canvas (zeroing dst = free guards /
    empty cells), DVE does the separable 3x3 max (colmax slab-local; rowmax lags
    one slab, reading boundary cm rows of neighbouring slabs), ACT makes the
    4B-aligned shifted copies.  16 fine slabs keep the pipeline ramp short.
  - Planar [E|O] column output; host deinterleaves + casts fp16 -> fp32.
"""
import os
import sys
import numpy as np
from contextlib import ExitStack

H, W = 256, 256
HP, WP = 128, 128
NCORES = 8
PPC = 128               # planes per core

NCHUNK = 64             # canvas chunks per plane (2 pair-rows each)
NIDX = 168              # compacted source slots per chunk (max valid = 163)
SEG = 130               # quadrant segment width (128 + 2 guard slots)
PAIRW = 4 * SEG         # 520 elements per canvas pair-row
NELEM = 2 * PAIRW       # 1040 elements per chunk
NSLAB = 16              # 16 slabs x 4 chunks
CPS = NCHUNK // NSLAB   # chunks per slab = 4
RPS = 2 * CPS           # pair-rows per slab = 8

for _p in ("/opt/trn_rl_repo", "/root/.axon_site/_ro/trn_rl_repo"):
    if os.path.isdir(_p) and _p not in sys.path:
        sys.path.append(_p)


def _build_nc():
    import concourse.bass as bass  # noqa: F401
    import concourse.tile as tile
    from concourse import bacc, mybir

    dt = mybir.dt.float16
    AO = mybir.AluOpType

    nc = bacc.Bacc("TRN2", target_bir_lowering=False, debug=False)
    v_in = nc.dram_tensor("vals", [PPC, NCHUNK, NIDX], dt, kind="ExternalInput").ap()
    ix_in = nc.dram_tensor("ix", [PPC, NCHUNK, NIDX], mybir.dt.int16,
                           kind="ExternalInput").ap()
    o_out = nc.dram_tensor("out", [PPC, H, W], dt, kind="ExternalOutput").ap()

    with tile.TileContext(nc) as tc, ExitStack() as ctx:
        pv = ctx.enter_context(tc.tile_pool(name="pv", bufs=2))
        pix = ctx.enter_context(tc.tile_pool(name="pix", bufs=2))
        pcv = ctx.enter_context(tc.tile_pool(name="pcv", bufs=2))
        psh = ctx.enter_context(tc.tile_pool(name="psh", bufs=2))
        pcm = ctx.enter_context(tc.tile_pool(name="pcm", bufs=3))
        pp = ctx.enter_context(tc.tile_pool(name="pp", bufs=2))
        pout = ctx.enter_context(tc.tile_pool(name="pout", bufs=2))

        cms = {}

        def scatter_slab(s):
            """local_scatter chunks 4s..4s+3 into one flat canvas tile."""
            VX = pv.tile([128, CPS, NIDX], dt, tag="VX")
            IX = pix.tile([128, CPS, NIDX], mybir.dt.int16, tag="IX")
            nc.sync.dma_start(VX[:], v_in[:, CPS * s:CPS * (s + 1), :])
            nc.sync.dma_start(IX[:], ix_in[:, CPS * s:CPS * (s + 1), :])
            CV = pcv.tile([128, CPS * NELEM], dt, tag="CV")
            for c in range(CPS):
                nc.gpsimd.local_scatter(
                    CV[:, c * NELEM:(c + 1) * NELEM], VX[:, c, :], IX[:, c, :],
                    channels=128, num_elems=NELEM, num_idxs=NIDX)
            return CV

        def colmax_slab(s, CV):
            """cm tiles [128,RPS,128]: row r <-> canvas pair-row RPS*s+r."""
            v = CV[:].rearrange("p (a g w) -> p a g w", a=RPS, g=4, w=SEG)
            E_e = v[:, :, 0, 0:128]
            E_o = v[:, :, 1, 0:128]
            O_e = v[:, :, 2, 2:130]
            O_o = v[:, :, 3, 2:130]
            sh_ee = psh.tile([128, RPS, 128], dt, tag="sh_ee")
            sh_eo = psh.tile([128, RPS, 128], dt, tag="sh_eo")
            sh_oe = psh.tile([128, RPS, 128], dt, tag="sh_oe")
            sh_oo = psh.tile([128, RPS, 128], dt, tag="sh_oo")
            nc.scalar.copy(sh_ee[:], v[:, :, 0, 1:129])
            nc.scalar.copy(sh_oe[:], v[:, :, 1, 1:129])
            nc.scalar.copy(sh_eo[:], v[:, :, 2, 1:129])
            nc.scalar.copy(sh_oo[:], v[:, :, 3, 1:129])
            cm_eE = pcm.tile([128, RPS, 128], dt, tag="cm_eE")
            cm_eO = pcm.tile([128, RPS, 128], dt, tag="cm_eO")
            cm_oE = pcm.tile([128, RPS, 128], dt, tag="cm_oE")
            cm_oO = pcm.tile([128, RPS, 128], dt, tag="cm_oO")
            P_e = pp.tile([128, RPS, 128], dt, tag="P_e")
            P_o = pp.tile([128, RPS, 128], dt, tag="P_o")
            nc.vector.tensor_tensor(P_e[:], E_e, O_e, AO.max)
            nc.vector.tensor_tensor(P_o[:], E_o, O_o, AO.max)
            nc.vector.tensor_tensor(cm_eE[:], P_e[:], sh_eo[:], AO.max)
            nc.vector.tensor_tensor(cm_eO[:], P_e[:], sh_ee[:], AO.max)
            nc.vector.tensor_tensor(cm_oE[:], P_o[:], sh_oo[:], AO.max)
            nc.vector.tensor_tensor(cm_oO[:], P_o[:], sh_oe[:], AO.max)
            cms[s] = (cm_eE, cm_eO, cm_oE, cm_oO)

        def rowmax_slab(s):
            """out rows [16s,16s+16): even r -> max(cm_o[r-1], S[r]),
            odd r -> max(S[r], cm_e[r+1]); S[r] = max(cm_e[r], cm_o[r])."""
            cm_eE, cm_eO, cm_oE, cm_oO = cms[s]
            prev = cms.get(s - 1)
            nxt = cms.get(s + 1)
            L = RPS - 1
            out_t = pout.tile([128, RPS, 2, 2, 128], dt, tag="out_t")
            S_E = pp.tile([128, RPS, 128], dt, tag="S_E")
            S_O = pp.tile([128, RPS, 128], dt, tag="S_O")
            nc.vector.tensor_tensor(S_E[:], cm_eE[:], cm_oE[:], AO.max)
            nc.vector.tensor_tensor(S_O[:], cm_eO[:], cm_oO[:], AO.max)
            nc.vector.tensor_tensor(out_t[:, 1:, 0, 0, :], S_E[:, 1:, :],
                                    cm_oE[:, 0:L, :], AO.max)
            nc.vector.tensor_tensor(out_t[:, 1:, 0, 1, :], S_O[:, 1:, :],
                                    cm_oO[:, 0:L, :], AO.max)
            nc.vector.tensor_tensor(out_t[:, 0:L, 1, 0, :], S_E[:, 0:L, :],
                                    cm_eE[:, 1:, :], AO.max)
            nc.vector.tensor_tensor(out_t[:, 0:L, 1, 1, :], S_O[:, 0:L, :],
                                    cm_eO[:, 1:, :], AO.max)
            if prev is None:
                nc.scalar.copy(out_t[:, 0, 0, 0, :], S_E[:, 0, :])
                nc.scalar.copy(out_t[:, 0, 0, 1, :], S_O[:, 0, :])
            else:
                nc.vector.tensor_tensor(out_t[:, 0, 0, 0, :], S_E[:, 0, :],
                                        prev[2][:, L, :], AO.max)
                nc.vector.tensor_tensor(out_t[:, 0, 0, 1, :], S_O[:, 0, :],
                                        prev[3][:, L, :], AO.max)
            if nxt is None:
                nc.scalar.copy(out_t[:, L, 1, 0, :], S_E[:, L, :])
                nc.scalar.copy(out_t[:, L, 1, 1, :], S_O[:, L, :])
            else:
                nc.vector.tensor_tensor(out_t[:, L, 1, 0, :], S_E[:, L, :],
                                        nxt[0][:, 0, :], AO.max)
                nc.vector.tensor_tensor(out_t[:, L, 1, 1, :], S_O[:, L, :],
                                        nxt[1][:, 0, :], AO.max)

            ov = o_out[:, 16 * s:16 * s + 16, :].rearrange(
                "p (r two) (x c) -> p r two x c", two=2, x=2)
            nc.sync.dma_start(ov, out_t[:])

        CV = scatter_slab(0)
        colmax_slab(0, CV)
        for s in range(1, NSLAB):
            CV = scatter_slab(s)
            colmax_slab(s, CV)
            rowmax_slab(s - 1)
        rowmax_slab(NSLAB - 1)

    nc.compile()
    return nc


_NC_CACHE = {}


def _get_nc():
    if "nc" not in _NC_CACHE:
        _NC_CACHE["nc"] = _build_nc()
    return _NC_CACHE["nc"]


def prepare_inputs(f, p):
    """Host prep: compacted collision-free (value, index) scatter lists.

    Returns (vals, idxs): both [N, NCHUNK, NIDX] (fp16 / int16).
    """
    N = f.shape[0] * f.shape[1]
    f2 = f.reshape(N, HP, WP)
    p2 = p.reshape(N, HP, WP).astype(np.int32)

    base = (np.arange(HP, dtype=np.int32)[:, None] * (2 * W)
            + np.arange(WP, dtype=np.int32)[None, :] * 2)
    d = p2 - base[None]
    dy = d >> 8
    dx = d & 255

    def sh(a, di, dj):
        out = np.full_like(a, -9)
        si0, si1 = max(di, 0), HP + min(di, 0)
        sj0, sj1 = max(dj, 0), WP + min(dj, 0)
        out[:, si0 - di:si1 - di, sj0 - dj:sj1 - dj] = a[:, si0:si1, sj0:sj1]
        return out

    dyR, dxR = sh(dy, 0, 1), sh(dx, 0, 1)
    dyD0, dxD0 = sh(dy, 1, 0), sh(dx, 1, 0)
    dyDm, dxDm = sh(dy, 1, -1), sh(dx, 1, -1)
    dyDp, dxDp = sh(dy, 1, 1), sh(dx, 1, 1)
    killed = ((dx == 2) & (dxR == 0) & (dyR == dy)) | ((dy == 2) & (
        ((dx == 0) & (dyDm == 0) & (dxDm == 2)) |
        ((dyD0 == 0) & (dxD0 == dx)) |
        ((dx == 2) & (dyDp == 0) & (dxDp == 0))))
    # Non-positive values can never win a window max (every 3x3 window of the
    # canvas contains an empty 0 cell), and their overwrite effect is already
    # captured by `killed` -- drop them from the scatter entirely.
    fp = f2.astype(np.float16)
    drop = killed | (fp <= 0)

    y = 2 * np.arange(HP, dtype=np.int32)[None, :, None] + dy
    x = 2 * np.arange(WP, dtype=np.int32)[None, None, :] + dx
    a = y >> 1
    seg = (x & 1) * 2 + (y & 1)              # [ee, oe, eo, oo]
    local_idx = (a & 1) * PAIRW + seg * SEG + np.where(x & 1 == 1, 2, 0) + (x >> 1)
    chunk_of = a >> 1

    # slot layout before compaction: 3 source rows (2t-1, 2t, 2t+1) x 128
    idx384 = np.full((N, NCHUNK, 384), -1, dtype=np.int16)
    val384 = np.zeros((N, NCHUNK, 384), dtype=np.float16)
    tt = np.arange(NCHUNK)
    r0 = np.maximum(0, 2 * tt - 1)
    for rloc in range(3):
        iv = r0 + rloc
        li = local_idx[:, iv, :]
        co = chunk_of[:, iv, :]
        kk = drop[:, iv, :]
        idx384[:, :, rloc * 128:(rloc + 1) * 128] = np.where(
            (co == tt[None, :, None]) & ~kk, li, -1).astype(np.int16)
        val384[:, :, rloc * 128:(rloc + 1) * 128] = fp[:, iv, :]

    # compact: valid slots first (stable), then truncate to NIDX
    invalid = idx384 < 0
    nvalid = (~invalid).sum(axis=-1)
    assert nvalid.max() <= NIDX, f"NIDX too small: need {nvalid.max()}"
    order = np.argsort(invalid, axis=-1, kind="stable")
    idxs = np.take_along_axis(idx384, order, axis=-1)[:, :, :NIDX]
    vals = np.take_along_axis(val384, order, axis=-1)[:, :, :NIDX]
    return np.ascontiguousarray(vals), np.ascontiguousarray(idxs)


def postprocess(out_planar):
    """[K, 256, 256] planar rows [E|O] -> interleaved columns, fp32."""
    res = np.empty(out_planar.shape, dtype=np.float32)
    res[..., 0::2] = out_planar[..., 0:128]
    res[..., 1::2] = out_planar[..., 128:256]
    return res


def kernel(**inputs):
    f = np.asarray(inputs["f"])
    p = np.asarray(inputs["provenance"])
    B, C = f.shape[:2]
    assert f.shape == (B, C, HP, WP) and B * C == NCORES * PPC

    vals, idxs = prepare_inputs(f, p)

    nc = _get_nc()
    from concourse.bass_utils import run_bass_kernel_spmd
    in_maps = [{"vals": vals[k * PPC:(k + 1) * PPC], "ix": idxs[k * PPC:(k + 1) * PPC]}
               for k in range(NCORES)]
    res = run_bass_kernel_spmd(nc, in_maps, core_ids=list(range(NCORES)))
    out = np.concatenate([postprocess(res.results[k]["out"]) for k in range(NCORES)],
                         axis=0)
    return out.reshape(B, C, H, W)


# revision 10
# speedup vs baseline: 1.1355x; 1.0962x over previous
"""Trainium2 Bass kernel for CudaMorphUnpool2D (max-unpool scatter + 3x3 dilation).

Strategy (v10):
  - 1024 (b,c) planes sharded 128/core across 8 NeuronCores (fully data parallel).
  - Host pre-computes, per pooled cell, its scatter target as a flat index into a
    chunked, parity-split canvas layout (64 chunks/plane, each chunk = 2 canvas
    pair-rows laid out as 4 quadrant segments [ee|oe|eo|oo] of 130 cols w/ guard
    slots).  Cells overwritten by a later raster writer (reference last-writer-
    wins semantics) are dropped, making indices collision-free; the surviving
    (value, index) pairs are compacted to NIDX slots per chunk (the per-index
    cost dominates GPSIMD LocalScatter: ~173 + 0.82/elem + 3.1/idx ns).
  - Device: GPSIMD local_scatter builds the canvas (zeroing dst = free guards /
    empty cells), DVE does the separable 3x3 max (colmax slab-local; rowmax lags
    one slab, reading boundary cm rows of neighbouring slabs), ACT makes the
    4B-aligned shifted copies.  16 fine slabs keep the pipeline ramp short.
  - Planar [E|O] column output; host deinterleaves + casts fp16 -> fp32.
"""
import os
import sys
import numpy as np
from contextlib import ExitStack

H, W = 256, 256
HP, WP = 128, 128
NCORES = 8
PPC = 128               # planes per core

NCHUNK = 64             # canvas chunks per plane (2 pair-rows each)
NIDX = 168              # compacted source slots per chunk (max valid = 163)
SEG = 130               # quadrant segment width (128 + 2 guard slots)
PAIRW = 4 * SEG         # 520 elements per canvas pair-row
NELEM = 2 * PAIRW       # 1040 elements per chunk
NSLAB = 16              # 16 slabs x 4 chunks
CPS = NCHUNK // NSLAB   # chunks per slab = 4
RPS = 2 * CPS           # pair-rows per slab = 8

for _p in ("/opt/trn_rl_repo", "/root/.axon_site/_ro/trn_rl_repo"):
    if os.path.isdir(_p) and _p not in sys.path:
        sys.path.append(_p)


def _build_nc():
    import concourse.bass as bass  # noqa: F401
    import concourse.tile as tile
    from concourse import bacc, mybir

    dt = mybir.dt.float16
    AO = mybir.AluOpType

    nc = bacc.Bacc("TRN2", target_bir_lowering=False, debug=False)
    v_in = nc.dram_tensor("vals", [PPC, NCHUNK, NIDX], dt, kind="ExternalInput").ap()
    ix_in = nc.dram_tensor("ix", [PPC, NCHUNK, NIDX], mybir.dt.int16,
                           kind="ExternalInput").ap()
    o_out = nc.dram_tensor("out", [PPC, H, W], dt, kind="ExternalOutput").ap()

    with tile.TileContext(nc) as tc, ExitStack() as ctx:
        pv = ctx.enter_context(tc.tile_pool(name="pv", bufs=1))
        pix = ctx.enter_context(tc.tile_pool(name="pix", bufs=1))
        pcv = ctx.enter_context(tc.tile_pool(name="pcv", bufs=2))
        psh = ctx.enter_context(tc.tile_pool(name="psh", bufs=2))
        pcm = ctx.enter_context(tc.tile_pool(name="pcm", bufs=3))
        pp = ctx.enter_context(tc.tile_pool(name="pp", bufs=2))
        pout = ctx.enter_context(tc.tile_pool(name="pout", bufs=2))

        cms = {}

        # all (val, idx) lists resident upfront: slab 0 per-chunk (early GPSIMD
        # start), the rest in one DMA each -- keeps input-DMA SBUF writes out of
        # the steady state where they contend with LocalScatter.
        VX_all = pv.tile([128, NCHUNK, NIDX], dt, tag="VX")
        IX_all = pix.tile([128, NCHUNK, NIDX], mybir.dt.int16, tag="IX")
        for c in range(CPS):
            nc.sync.dma_start(VX_all[:, c, :], v_in[:, c, :])
            nc.sync.dma_start(IX_all[:, c, :], ix_in[:, c, :])
        nc.sync.dma_start(VX_all[:, CPS:, :], v_in[:, CPS:, :])
        nc.sync.dma_start(IX_all[:, CPS:, :], ix_in[:, CPS:, :])

        def scatter_slab(s):
            """local_scatter chunks 4s..4s+3 into one flat canvas tile."""
            CV = pcv.tile([128, CPS * NELEM], dt, tag="CV")
            for c in range(CPS):
                t = CPS * s + c
                nc.gpsimd.local_scatter(
                    CV[:, c * NELEM:(c + 1) * NELEM], VX_all[:, t, :],
                    IX_all[:, t, :],
                    channels=128, num_elems=NELEM, num_idxs=NIDX)
            return CV

        def colmax_slab(s, CV):
            """cm tiles [128,RPS,128]: row r <-> canvas pair-row RPS*s+r."""
            v = CV[:].rearrange("p (a g w) -> p a g w", a=RPS, g=4, w=SEG)
            E_e = v[:, :, 0, 0:128]
            E_o = v[:, :, 1, 0:128]
            O_e = v[:, :, 2, 2:130]
            O_o = v[:, :, 3, 2:130]
            sh_ee = psh.tile([128, RPS, 128], dt, tag="sh_ee")
            sh_eo = psh.tile([128, RPS, 128], dt, tag="sh_eo")
            sh_oe = psh.tile([128, RPS, 128], dt, tag="sh_oe")
            sh_oo = psh.tile([128, RPS, 128], dt, tag="sh_oo")
            nc.scalar.copy(sh_ee[:], v[:, :, 0, 1:129])
            nc.scalar.copy(sh_oe[:], v[:, :, 1, 1:129])
            nc.scalar.copy(sh_eo[:], v[:, :, 2, 1:129])
            nc.scalar.copy(sh_oo[:], v[:, :, 3, 1:129])
            cm_eE = pcm.tile([128, RPS, 128], dt, tag="cm_eE")
            cm_eO = pcm.tile([128, RPS, 128], dt, tag="cm_eO")
            cm_oE = pcm.tile([128, RPS, 128], dt, tag="cm_oE")
            cm_oO = pcm.tile([128, RPS, 128], dt, tag="cm_oO")
            P_e = pp.tile([128, RPS, 128], dt, tag="P_e")
            P_o = pp.tile([128, RPS, 128], dt, tag="P_o")
            nc.vector.tensor_tensor(P_e[:], E_e, O_e, AO.max)
            nc.vector.tensor_tensor(P_o[:], E_o, O_o, AO.max)
            nc.vector.tensor_tensor(cm_eE[:], P_e[:], sh_eo[:], AO.max)
            nc.vector.tensor_tensor(cm_eO[:], P_e[:], sh_ee[:], AO.max)
            nc.vector.tensor_tensor(cm_oE[:], P_o[:], sh_oo[:], AO.max)
            nc.vector.tensor_tensor(cm_oO[:], P_o[:], sh_oe[:], AO.max)
            cms[s] = (cm_eE, cm_eO, cm_oE, cm_oO)

        def rowmax_slab(s):
            """out rows [16s,16s+16): even r -> max(cm_o[r-1], S[r]),
            odd r -> max(S[r], cm_e[r+1]); S[r] = max(cm_e[r], cm_o[r])."""
            cm_eE, cm_eO, cm_oE, cm_oO = cms[s]
            prev = cms.get(s - 1)
            nxt = cms.get(s + 1)
            L = RPS - 1
            out_t = pout.tile([128, RPS, 2, 2, 128], dt, tag="out_t")
            S_E = pp.tile([128, RPS, 128], dt, tag="S_E")
            S_O = pp.tile([128, RPS, 128], dt, tag="S_O")
            nc.vector.tensor_tensor(S_E[:], cm_eE[:], cm_oE[:], AO.max)
            nc.vector.tensor_tensor(S_O[:], cm_eO[:], cm_oO[:], AO.max)
            nc.vector.tensor_tensor(out_t[:, 1:, 0, 0, :], S_E[:, 1:, :],
                                    cm_oE[:, 0:L, :], AO.max)
            nc.vector.tensor_tensor(out_t[:, 1:, 0, 1, :], S_O[:, 1:, :],
                                    cm_oO[:, 0:L, :], AO.max)
            nc.vector.tensor_tensor(out_t[:, 0:L, 1, 0, :], S_E[:, 0:L, :],
                                    cm_eE[:, 1:, :], AO.max)
            nc.vector.tensor_tensor(out_t[:, 0:L, 1, 1, :], S_O[:, 0:L, :],
                                    cm_eO[:, 1:, :], AO.max)
            if prev is None:
                nc.scalar.copy(out_t[:, 0, 0, 0, :], S_E[:, 0, :])
                nc.scalar.copy(out_t[:, 0, 0, 1, :], S_O[:, 0, :])
            else:
                nc.vector.tensor_tensor(out_t[:, 0, 0, 0, :], S_E[:, 0, :],
                                        prev[2][:, L, :], AO.max)
                nc.vector.tensor_tensor(out_t[:, 0, 0, 1, :], S_O[:, 0, :],
                                        prev[3][:, L, :], AO.max)
            if nxt is None:
                nc.scalar.copy(out_t[:, L, 1, 0, :], S_E[:, L, :])
                nc.scalar.copy(out_t[:, L, 1, 1, :], S_O[:, L, :])
            else:
                nc.vector.tensor_tensor(out_t[:, L, 1, 0, :], S_E[:, L, :],
                                        nxt[0][:, 0, :], AO.max)
                nc.vector.tensor_tensor(out_t[:, L, 1, 1, :], S_O[:, L, :],
                                        nxt[1][:, 0, :], AO.max)

            hR = RPS // 2
            ov = o_out[:, 16 * s:16 * s + 16, :].rearrange(
                "p (r two) (x c) -> p r two x c", two=2, x=2)
            nc.sync.dma_start(ov[:, 0:hR], out_t[:, 0:hR])
            nc.sync.dma_start(ov[:, hR:], out_t[:, hR:])

        CV = scatter_slab(0)
        colmax_slab(0, CV)
        for s in range(1, NSLAB):
            CV = scatter_slab(s)
            colmax_slab(s, CV)
            rowmax_slab(s - 1)
        rowmax_slab(NSLAB - 1)

    nc.compile()
    return nc


_NC_CACHE = {}


def _get_nc():
    if "nc" not in _NC_CACHE:
        _NC_CACHE["nc"] = _build_nc()
    return _NC_CACHE["nc"]


def prepare_inputs(f, p):
    """Host prep: compacted collision-free (value, index) scatter lists.

    Returns (vals, idxs): both [N, NCHUNK, NIDX] (fp16 / int16).
    """
    N = f.shape[0] * f.shape[1]
    f2 = f.reshape(N, HP, WP)
    p2 = p.reshape(N, HP, WP).astype(np.int32)

    base = (np.arange(HP, dtype=np.int32)[:, None] * (2 * W)
            + np.arange(WP, dtype=np.int32)[None, :] * 2)
    d = p2 - base[None]
    dy = d >> 8
    dx = d & 255

    def sh(a, di, dj):
        out = np.full_like(a, -9)
        si0, si1 = max(di, 0), HP + min(di, 0)
        sj0, sj1 = max(dj, 0), WP + min(dj, 0)
        out[:, si0 - di:si1 - di, sj0 - dj:sj1 - dj] = a[:, si0:si1, sj0:sj1]
        return out

    dyR, dxR = sh(dy, 0, 1), sh(dx, 0, 1)
    dyD0, dxD0 = sh(dy, 1, 0), sh(dx, 1, 0)
    dyDm, dxDm = sh(dy, 1, -1), sh(dx, 1, -1)
    dyDp, dxDp = sh(dy, 1, 1), sh(dx, 1, 1)
    killed = ((dx == 2) & (dxR == 0) & (dyR == dy)) | ((dy == 2) & (
        ((dx == 0) & (dyDm == 0) & (dxDm == 2)) |
        ((dyD0 == 0) & (dxD0 == dx)) |
        ((dx == 2) & (dyDp == 0) & (dxDp == 0))))
    # Non-positive values can never win a window max (every 3x3 window of the
    # canvas contains an empty 0 cell), and their overwrite effect is already
    # captured by `killed` -- drop them from the scatter entirely.
    fp = f2.astype(np.float16)
    drop = killed | (fp <= 0)

    y = 2 * np.arange(HP, dtype=np.int32)[None, :, None] + dy
    x = 2 * np.arange(WP, dtype=np.int32)[None, None, :] + dx
    a = y >> 1
    seg = (x & 1) * 2 + (y & 1)              # [ee, oe, eo, oo]
    local_idx = (a & 1) * PAIRW + seg * SEG + np.where(x & 1 == 1, 2, 0) + (x >> 1)
    chunk_of = a >> 1

    # slot layout before compaction: 3 source rows (2t-1, 2t, 2t+1) x 128
    idx384 = np.full((N, NCHUNK, 384), -1, dtype=np.int16)
    val384 = np.zeros((N, NCHUNK, 384), dtype=np.float16)
    tt = np.arange(NCHUNK)
    r0 = np.maximum(0, 2 * tt - 1)
    for rloc in range(3):
        iv = r0 + rloc
        li = local_idx[:, iv, :]
        co = chunk_of[:, iv, :]
        kk = drop[:, iv, :]
        idx384[:, :, rloc * 128:(rloc + 1) * 128] = np.where(
            (co == tt[None, :, None]) & ~kk, li, -1).astype(np.int16)
        val384[:, :, rloc * 128:(rloc + 1) * 128] = fp[:, iv, :]

    # compact: valid slots first (stable), then truncate to NIDX
    invalid = idx384 < 0
    nvalid = (~invalid).sum(axis=-1)
    assert nvalid.max() <= NIDX, f"NIDX too small: need {nvalid.max()}"
    order = np.argsort(invalid, axis=-1, kind="stable")
    idxs = np.take_along_axis(idx384, order, axis=-1)[:, :, :NIDX]
    vals = np.take_along_axis(val384, order, axis=-1)[:, :, :NIDX]
    return np.ascontiguousarray(vals), np.ascontiguousarray(idxs)


def postprocess(out_planar):
    """[K, 256, 256] planar rows [E|O] -> interleaved columns, fp32."""
    res = np.empty(out_planar.shape, dtype=np.float32)
    res[..., 0::2] = out_planar[..., 0:128]
    res[..., 1::2] = out_planar[..., 128:256]
    return res


def kernel(**inputs):
    f = np.asarray(inputs["f"])
    p = np.asarray(inputs["provenance"])
    B, C = f.shape[:2]
    assert f.shape == (B, C, HP, WP) and B * C == NCORES * PPC

    vals, idxs = prepare_inputs(f, p)

    nc = _get_nc()
    from concourse.bass_utils import run_bass_kernel_spmd
    in_maps = [{"vals": vals[k * PPC:(k + 1) * PPC], "ix": idxs[k * PPC:(k + 1) * PPC]}
               for k in range(NCORES)]
    res = run_bass_kernel_spmd(nc, in_maps, core_ids=list(range(NCORES)))
    out = np.concatenate([postprocess(res.results[k]["out"]) for k in range(NCORES)],
                         axis=0)
    return out.reshape(B, C, H, W)


# revision 12
# speedup vs baseline: 1.1470x; 1.0101x over previous
"""Trainium2 Bass kernel for CudaMorphUnpool2D (max-unpool scatter + 3x3 dilation).

Strategy (v10):
  - 1024 (b,c) planes sharded 128/core across 8 NeuronCores (fully data parallel).
  - Host pre-computes, per pooled cell, its scatter target as a flat index into a
    chunked, parity-split canvas layout (64 chunks/plane, each chunk = 2 canvas
    pair-rows laid out as 4 quadrant segments [ee|oe|eo|oo] of 130 cols w/ guard
    slots).  Cells overwritten by a later raster writer (reference last-writer-
    wins semantics) are dropped, making indices collision-free; the surviving
    (value, index) pairs are compacted to NIDX slots per chunk (the per-index
    cost dominates GPSIMD LocalScatter: ~173 + 0.82/elem + 3.1/idx ns).
  - Device: GPSIMD local_scatter builds the canvas (zeroing dst = free guards /
    empty cells), DVE does the separable 3x3 max (colmax slab-local; rowmax lags
    one slab, reading boundary cm rows of neighbouring slabs), ACT makes the
    4B-aligned shifted copies.  16 fine slabs keep the pipeline ramp short.
  - Planar [E|O] column output; host deinterleaves + casts fp16 -> fp32.
"""
import os
import sys
import numpy as np
from contextlib import ExitStack

H, W = 256, 256
HP, WP = 128, 128
NCORES = 8
PPC = 128               # planes per core

NCHUNK = 64             # canvas chunks per plane (2 pair-rows each)
NIDX = 168              # compacted source slots per chunk (max valid = 163)
SEG = 130               # quadrant segment width (128 + 2 guard slots)
PAIRW = 4 * SEG         # 520 elements per canvas pair-row
NELEM = 2 * PAIRW       # 1040 elements per chunk
NSLAB = 16              # 16 slabs x 4 chunks
CPS = NCHUNK // NSLAB   # chunks per slab = 4
RPS = 2 * CPS           # pair-rows per slab = 8

for _p in ("/opt/trn_rl_repo", "/root/.axon_site/_ro/trn_rl_repo"):
    if os.path.isdir(_p) and _p not in sys.path:
        sys.path.append(_p)


def _build_nc():
    import concourse.bass as bass  # noqa: F401
    import concourse.tile as tile
    from concourse import bacc, mybir

    dt = mybir.dt.float16
    AO = mybir.AluOpType

    nc = bacc.Bacc("TRN2", target_bir_lowering=False, debug=False)
    v_in = nc.dram_tensor("vals", [PPC, NCHUNK, NIDX], dt, kind="ExternalInput").ap()
    ix_in = nc.dram_tensor("ix", [PPC, NCHUNK, NIDX], mybir.dt.int16,
                           kind="ExternalInput").ap()
    o_out = nc.dram_tensor("out", [PPC, H, W], dt, kind="ExternalOutput").ap()

    with tile.TileContext(nc) as tc, ExitStack() as ctx:
        pv = ctx.enter_context(tc.tile_pool(name="pv", bufs=1))
        pix = ctx.enter_context(tc.tile_pool(name="pix", bufs=1))
        pcv = ctx.enter_context(tc.tile_pool(name="pcv", bufs=2))
        psh = ctx.enter_context(tc.tile_pool(name="psh", bufs=2))
        pcm = ctx.enter_context(tc.tile_pool(name="pcm", bufs=3))
        pp = ctx.enter_context(tc.tile_pool(name="pp", bufs=2))
        pout = ctx.enter_context(tc.tile_pool(name="pout", bufs=2))

        cms = {}

        # all (val, idx) lists resident upfront: slab 0 per-chunk (early GPSIMD
        # start), the rest in one DMA each -- keeps input-DMA SBUF writes out of
        # the steady state where they contend with LocalScatter.
        VX_all = pv.tile([128, NCHUNK, NIDX], dt, tag="VX")
        IX_all = pix.tile([128, NCHUNK, NIDX], mybir.dt.int16, tag="IX")
        for c in range(CPS):
            nc.sync.dma_start(VX_all[:, c, :], v_in[:, c, :])
            nc.sync.dma_start(IX_all[:, c, :], ix_in[:, c, :])
        nc.sync.dma_start(VX_all[:, CPS:, :], v_in[:, CPS:, :])
        nc.sync.dma_start(IX_all[:, CPS:, :], ix_in[:, CPS:, :])

        def scatter_slab(s):
            """local_scatter chunks 4s..4s+3 into one flat canvas tile."""
            CV = pcv.tile([128, CPS * NELEM], dt, tag="CV")
            for c in range(CPS):
                t = CPS * s + c
                nc.gpsimd.local_scatter(
                    CV[:, c * NELEM:(c + 1) * NELEM], VX_all[:, t, :],
                    IX_all[:, t, :],
                    channels=128, num_elems=NELEM, num_idxs=NIDX)
            return CV

        def colmax_slab(s, CV):
            """cm tiles [128,RPS,128]: row r <-> canvas pair-row RPS*s+r."""
            v = CV[:].rearrange("p (a g w) -> p a g w", a=RPS, g=4, w=SEG)
            E_e = v[:, :, 0, 0:128]
            E_o = v[:, :, 1, 0:128]
            O_e = v[:, :, 2, 2:130]
            O_o = v[:, :, 3, 2:130]
            sh_ee = psh.tile([128, RPS, 128], dt, tag="sh_ee")
            sh_eo = psh.tile([128, RPS, 128], dt, tag="sh_eo")
            sh_oe = psh.tile([128, RPS, 128], dt, tag="sh_oe")
            sh_oo = psh.tile([128, RPS, 128], dt, tag="sh_oo")
            nc.scalar.copy(sh_ee[:], v[:, :, 0, 1:129])
            nc.scalar.copy(sh_oe[:], v[:, :, 1, 1:129])
            nc.scalar.copy(sh_eo[:], v[:, :, 2, 1:129])
            nc.scalar.copy(sh_oo[:], v[:, :, 3, 1:129])
            cm_eE = pcm.tile([128, RPS, 128], dt, tag="cm_eE")
            cm_eO = pcm.tile([128, RPS, 128], dt, tag="cm_eO")
            cm_oE = pcm.tile([128, RPS, 128], dt, tag="cm_oE")
            cm_oO = pcm.tile([128, RPS, 128], dt, tag="cm_oO")
            P_e = pp.tile([128, RPS, 128], dt, tag="P_e")
            P_o = pp.tile([128, RPS, 128], dt, tag="P_o")
            nc.vector.tensor_tensor(P_e[:], E_e, O_e, AO.max)
            nc.vector.tensor_tensor(P_o[:], E_o, O_o, AO.max)
            nc.vector.tensor_tensor(cm_eE[:], P_e[:], sh_eo[:], AO.max)
            nc.vector.tensor_tensor(cm_eO[:], P_e[:], sh_ee[:], AO.max)
            nc.vector.tensor_tensor(cm_oE[:], P_o[:], sh_oo[:], AO.max)
            nc.vector.tensor_tensor(cm_oO[:], P_o[:], sh_oe[:], AO.max)
            cms[s] = (cm_eE, cm_eO, cm_oE, cm_oO)

        def rowmax_slab(s):
            """out rows [16s,16s+16): even r -> max(cm_o[r-1], S[r]),
            odd r -> max(S[r], cm_e[r+1]); S[r] = max(cm_e[r], cm_o[r])."""
            cm_eE, cm_eO, cm_oE, cm_oO = cms[s]
            prev = cms.get(s - 1)
            nxt = cms.get(s + 1)
            L = RPS - 1
            out_t = pout.tile([128, RPS, 2, 2, 128], dt, tag="out_t")
            S_E = pp.tile([128, RPS, 128], dt, tag="S_E")
            S_O = pp.tile([128, RPS, 128], dt, tag="S_O")
            nc.vector.tensor_tensor(S_E[:], cm_eE[:], cm_oE[:], AO.max)
            nc.vector.tensor_tensor(S_O[:], cm_eO[:], cm_oO[:], AO.max)
            nc.vector.tensor_tensor(out_t[:, 1:, 0, 0, :], S_E[:, 1:, :],
                                    cm_oE[:, 0:L, :], AO.max)
            nc.vector.tensor_tensor(out_t[:, 1:, 0, 1, :], S_O[:, 1:, :],
                                    cm_oO[:, 0:L, :], AO.max)
            nc.vector.tensor_tensor(out_t[:, 0:L, 1, 0, :], S_E[:, 0:L, :],
                                    cm_eE[:, 1:, :], AO.max)
            nc.vector.tensor_tensor(out_t[:, 0:L, 1, 1, :], S_O[:, 0:L, :],
                                    cm_eO[:, 1:, :], AO.max)
            if prev is None:
                nc.scalar.copy(out_t[:, 0, 0, 0, :], S_E[:, 0, :])
                nc.scalar.copy(out_t[:, 0, 0, 1, :], S_O[:, 0, :])
            else:
                nc.vector.tensor_tensor(out_t[:, 0, 0, 0, :], S_E[:, 0, :],
                                        prev[2][:, L, :], AO.max)
                nc.vector.tensor_tensor(out_t[:, 0, 0, 1, :], S_O[:, 0, :],
                                        prev[3][:, L, :], AO.max)
            if nxt is None:
                nc.scalar.copy(out_t[:, L, 1, 0, :], S_E[:, L, :])
                nc.scalar.copy(out_t[:, L, 1, 1, :], S_O[:, L, :])
            else:
                nc.vector.tensor_tensor(out_t[:, L, 1, 0, :], S_E[:, L, :],
                                        nxt[0][:, 0, :], AO.max)
                nc.vector.tensor_tensor(out_t[:, L, 1, 1, :], S_O[:, L, :],
                                        nxt[1][:, 0, :], AO.max)

            hR = RPS // 2
            ov = o_out[:, 16 * s:16 * s + 16, :].rearrange(
                "p (r two) (x c) -> p r two x c", two=2, x=2)
            nc.sync.dma_start(ov[:, 0:hR], out_t[:, 0:hR])
            nc.sync.dma_start(ov[:, hR:], out_t[:, hR:])

        CV = scatter_slab(0)
        colmax_slab(0, CV)
        for s in range(1, NSLAB):
            CV = scatter_slab(s)
            colmax_slab(s, CV)
            rowmax_slab(s - 1)
        rowmax_slab(NSLAB - 1)

    nc.compile()
    return nc


_NC_CACHE = {}


def _get_nc():
    if "nc" not in _NC_CACHE:
        _NC_CACHE["nc"] = _build_nc()
    return _NC_CACHE["nc"]


def prepare_inputs(f, p):
    """Host prep: compacted collision-free (value, index) scatter lists.

    Returns (vals, idxs): both [N, NCHUNK, NIDX] (fp16 / int16).
    """
    N = f.shape[0] * f.shape[1]
    f2 = f.reshape(N, HP, WP)
    p2 = p.reshape(N, HP, WP).astype(np.int32)

    base = (np.arange(HP, dtype=np.int32)[:, None] * (2 * W)
            + np.arange(WP, dtype=np.int32)[None, :] * 2)
    d = p2 - base[None]
    dy = d >> 8
    dx = d & 255

    def sh(a, di, dj):
        out = np.full_like(a, -9)
        si0, si1 = max(di, 0), HP + min(di, 0)
        sj0, sj1 = max(dj, 0), WP + min(dj, 0)
        out[:, si0 - di:si1 - di, sj0 - dj:sj1 - dj] = a[:, si0:si1, sj0:sj1]
        return out

    dyR, dxR = sh(dy, 0, 1), sh(dx, 0, 1)
    dyD0, dxD0 = sh(dy, 1, 0), sh(dx, 1, 0)
    dyDm, dxDm = sh(dy, 1, -1), sh(dx, 1, -1)
    dyDp, dxDp = sh(dy, 1, 1), sh(dx, 1, 1)
    killed = ((dx == 2) & (dxR == 0) & (dyR == dy)) | ((dy == 2) & (
        ((dx == 0) & (dyDm == 0) & (dxDm == 2)) |
        ((dyD0 == 0) & (dxD0 == dx)) |
        ((dx == 2) & (dyDp == 0) & (dxDp == 0))))
    # Non-positive values can never win a window max (every 3x3 window of the
    # canvas contains an empty 0 cell), and their overwrite effect is already
    # captured by `killed` -- drop them from the scatter entirely.
    fp = f2.astype(np.float16)
    drop = killed | (fp <= 0)

    y = 2 * np.arange(HP, dtype=np.int32)[None, :, None] + dy
    x = 2 * np.arange(WP, dtype=np.int32)[None, None, :] + dx
    a = y >> 1
    seg = (x & 1) * 2 + (y & 1)              # [ee, oe, eo, oo]
    local_idx = (a & 1) * PAIRW + seg * SEG + np.where(x & 1 == 1, 2, 0) + (x >> 1)
    chunk_of = a >> 1

    # slot layout before compaction: 3 source rows (2t-1, 2t, 2t+1) x 128
    idx384 = np.full((N, NCHUNK, 384), -1, dtype=np.int16)
    val384 = np.zeros((N, NCHUNK, 384), dtype=np.float16)
    tt = np.arange(NCHUNK)
    r0 = np.maximum(0, 2 * tt - 1)
    for rloc in range(3):
        iv = r0 + rloc
        li = local_idx[:, iv, :]
        co = chunk_of[:, iv, :]
        kk = drop[:, iv, :]
        idx384[:, :, rloc * 128:(rloc + 1) * 128] = np.where(
            (co == tt[None, :, None]) & ~kk, li, -1).astype(np.int16)
        val384[:, :, rloc * 128:(rloc + 1) * 128] = fp[:, iv, :]

    # compact: valid slots first (stable), then truncate to NIDX
    invalid = idx384 < 0
    nvalid = (~invalid).sum(axis=-1)
    assert nvalid.max() <= NIDX, f"NIDX too small: need {nvalid.max()}"
    order = np.argsort(invalid, axis=-1, kind="stable")
    idxs = np.take_along_axis(idx384, order, axis=-1)[:, :, :NIDX]
    vals = np.take_along_axis(val384, order, axis=-1)[:, :, :NIDX]
    return np.ascontiguousarray(vals), np.ascontiguousarray(idxs)


def postprocess(out_planar):
    """[K, 256, 256] planar rows [E|O] -> interleaved columns, fp32."""
    res = np.empty(out_planar.shape, dtype=np.float32)
    res[..., 0::2] = out_planar[..., 0:128]
    res[..., 1::2] = out_planar[..., 128:256]
    return res


def kernel(**inputs):
    f = np.asarray(inputs["f"])
    p = np.asarray(inputs["provenance"])
    B, C = f.shape[:2]
    assert f.shape == (B, C, HP, WP) and B * C == NCORES * PPC

    vals, idxs = prepare_inputs(f, p)

    nc = _get_nc()
    from concourse.bass_utils import run_bass_kernel_spmd
    in_maps = [{"vals": vals[k * PPC:(k + 1) * PPC], "ix": idxs[k * PPC:(k + 1) * PPC]}
               for k in range(NCORES)]
    res = run_bass_kernel_spmd(nc, in_maps, core_ids=list(range(NCORES)))
    out = np.concatenate([postprocess(res.results[k]["out"]) for k in range(NCORES)],
                         axis=0)
    return out.reshape(B, C, H, W)
